# revision 1
# baseline (speedup 1.0000x reference)
"""DGS2D bilinear-sample + analytic-spatial-derivative layer on 8 TRN2 cores.

Contract: kernel(**inputs) takes the FULL inputs of nn_DGS2DLayer
  input  [4, 32, 512, 512] f32, grid [4, 65536, 3] f32,
  fScaleWidth [4] f32, fScaleHeight [4] f32
and returns the FULL output [4, 32, 4, 65536] f32.

Sharding (host): 2 cores per batch element; each core takes half the queries.
Host-side prep is layout-only (no FLOPs): the feature map is transposed to
channel-minor and expanded into a 4-corner stencil table
    feat2[i*W+j] = [feat[i,j,:], feat[i+1,j,:], feat[i,j+1,:], feat[i+1,j+1,:]]
(512B rows) so that ONE indirect-DMA descriptor per query fetches all four
bilinear corners at DMA line rate.

Device kernel (per core): coordinate math -> int32 row indices -> 256
indirect gathers ([P,1] offset APs - the only HW-supported form) ->
bilinear interpolation + analytic derivatives + camera-space chain rule
(fp16 intermediates, fp32 gather/coords) -> fp16->f32 cast stores with
512B-contiguous DRAM runs.
"""
import numpy as np

import concourse.bacc as bacc
import concourse.bass as bass
import concourse.mybir as mybir
import concourse.tile as tile

P = 128
F32 = mybir.dt.float32
F16 = mybir.dt.float16
I32 = mybir.dt.int32
Alu = mybir.AluOpType

B, C, H, W, Q = 4, 32, 512, 512, 65536
NCORES = 8
QC = Q // (NCORES // B)   # queries per core (2 cores per batch)

# fp16 intermediates: ~1e-3 rel err vs reference (f32: ~3e-4, ~25% slower).
FP16_COMPUTE = True


def _build_kernel(H=H, W=W, C=C, QC=QC, n_g=32, CHUNK=128,
                  fp16=FP16_COMPUTE, expand_act=True, repeat=1,
                  pool_ops=False, act_out=False, hi16=False):
    S = QC // P
    n_chunks = S // CHUNK
    tiles_per_chunk = CHUNK // n_g
    NR = (H - 1) * W
    D2 = 2 * C
    GE = 4 * C
    CDT = F16 if fp16 else F32
    ODT = F16
    half_w = 0.5 * (W - 1)
    half_h = 0.5 * (H - 1)

    nc = bacc.Bacc("TRN2", target_bir_lowering=False, debug=False)

    feat2 = nc.dram_tensor("feat2", [NR, GE], F32, kind="ExternalInput")
    grid_q = nc.dram_tensor("grid_q", [QC, 3], F32, kind="ExternalInput")
    fsw = nc.dram_tensor("fsw", [1, 1], F32, kind="ExternalInput")
    fsh = nc.dram_tensor("fsh", [1, 1], F32, kind="ExternalInput")
    out = nc.dram_tensor("out", [C, 4, QC], F32, kind="ExternalOutput")

    with tile.TileContext(nc) as tc:
        with (
            tc.tile_pool(name="setup", bufs=1) as sp,
            tc.tile_pool(name="gp", bufs=2 if fp16 else 1) as gp,
            tc.tile_pool(name="lp", bufs=2) as lp,
            tc.tile_pool(name="ep", bufs=2 if fp16 else 1) as ep,
            tc.tile_pool(name="tp", bufs=2 if fp16 else 1) as tp,
            tc.tile_pool(name="op", bufs=2 if fp16 else 1) as op,
        ):
            # ---------------- setup: per-query coordinate arrays ------------
            grid_sb = sp.tile([P, S, 3], F32)
            nc.sync.dma_start(
                grid_sb[:], grid_q[:].rearrange("(p s) t -> p s t", p=P))
            fw_sb = sp.tile([1, 1], F32)
            nc.sync.dma_start(fw_sb[:], fsw[:])
            fh_sb = sp.tile([1, 1], F32)
            nc.sync.dma_start(fh_sb[:], fsh[:])

            xv = grid_sb[:, :, 0]
            yv = grid_sb[:, :, 1]
            zv = grid_sb[:, :, 2]

            jx = sp.tile([P, S], F32)
            nc.vector.tensor_scalar(out=jx[:], in0=xv, scalar1=1.0,
                                    scalar2=half_w, op0=Alu.add, op1=Alu.mult)
            iy = sp.tile([P, S], F32)
            nc.vector.tensor_scalar(out=iy[:], in0=yv, scalar1=1.0,
                                    scalar2=half_h, op0=Alu.add, op1=Alu.mult)

            # floor(v) = round(v) - (round(v) > v): HW f32->i32 cast is
            # round-to-nearest-even; is_gt corrects upward rounds.
            def floor_of(v, nm):
                ri = sp.tile([P, S], I32, name=f"ri_{nm}")
                nc.vector.tensor_copy(ri[:], v)
                rf = sp.tile([P, S], F32, name=f"rf_{nm}")
                nc.vector.tensor_copy(rf[:], ri[:])
                mk = sp.tile([P, S], F32, name=f"mk_{nm}")
                nc.vector.tensor_tensor(out=mk[:], in0=rf[:], in1=v,
                                        op=Alu.is_gt)
                fl = sp.tile([P, S], F32, name=f"fl_{nm}")
                nc.vector.tensor_tensor(out=fl[:], in0=rf[:], in1=mk[:],
                                        op=Alu.subtract)
                return fl

            j0 = floor_of(jx[:], "jx")
            i0 = floor_of(iy[:], "iy")
            txf = sp.tile([P, S], F32)
            nc.vector.tensor_tensor(out=txf[:], in0=jx[:], in1=j0[:],
                                    op=Alu.subtract)
            tyf = sp.tile([P, S], F32)
            nc.vector.tensor_tensor(out=tyf[:], in0=iy[:], in1=i0[:],
                                    op=Alu.subtract)
            idxf = sp.tile([P, S], F32)
            nc.vector.scalar_tensor_tensor(out=idxf[:], in0=i0[:],
                                           scalar=float(W), in1=j0[:],
                                           op0=Alu.mult, op1=Alu.add)
            idx_t = sp.tile([P, S], I32)
            nc.vector.tensor_copy(idx_t[:], idxf[:])

            zinv = sp.tile([P, S], F32)
            nc.vector.reciprocal(zinv[:], zv)

            fwb = sp.tile([P, 1], F32)
            nc.gpsimd.partition_broadcast(fwb[:], fw_sb[:])
            fhb = sp.tile([P, 1], F32)
            nc.gpsimd.partition_broadcast(fhb[:], fh_sb[:])
            fws = sp.tile([P, 1], F32)
            nc.vector.tensor_scalar(out=fws[:], in0=fwb[:], scalar1=half_w,
                                    scalar2=None, op0=Alu.mult)
            fhs = sp.tile([P, 1], F32)
            nc.vector.tensor_scalar(out=fhs[:], in0=fhb[:], scalar1=half_h,
                                    scalar2=None, op0=Alu.mult)

            tx_c = sp.tile([P, S], CDT)
            nc.scalar.copy(tx_c[:], txf[:])
            ty_c = sp.tile([P, S], CDT)
            nc.scalar.copy(ty_c[:], tyf[:])
            ax_c = sp.tile([P, S], CDT)
            nc.vector.tensor_scalar(out=ax_c[:], in0=zinv[:], scalar1=fws[:],
                                    scalar2=None, op0=Alu.mult)
            ay_c = sp.tile([P, S], CDT)
            nc.vector.tensor_scalar(out=ay_c[:], in0=zinv[:], scalar1=fhs[:],
                                    scalar2=None, op0=Alu.mult)
            czx_c = sp.tile([P, S], CDT)
            nc.vector.scalar_tensor_tensor(out=czx_c[:], in0=xv,
                                           scalar=-half_w, in1=zinv[:],
                                           op0=Alu.mult, op1=Alu.mult)
            czy_c = sp.tile([P, S], CDT)
            nc.vector.scalar_tensor_tensor(out=czy_c[:], in0=yv,
                                           scalar=-half_h, in1=zinv[:],
                                           op0=Alu.mult, op1=Alu.mult)

            # ---------------- main loop ------------------------------------
            for rep, ci in ((r, c) for r in range(repeat)
                            for c in range(n_chunks)):
                osb = {}
                for k in range(4):
                    osb[k] = op.tile([P, C, CHUNK], ODT, tag=f"o{k}",
                                     name=f"o{k}_{rep}_{ci}")
                for ti in range(tiles_per_chunk):
                    g = ci * tiles_per_chunk + ti
                    gs, ge = g * n_g, (g + 1) * n_g
                    rs, re = ti * n_g, (ti + 1) * n_g

                    gt = gp.tile([P, n_g, GE], F32, tag="G")
                    # HW contract: ONE index per partition per indirect DMA.
                    for s in range(n_g):
                        nc.gpsimd.indirect_dma_start(
                            out=gt[:, s, :], out_offset=None, in_=feat2[:],
                            in_offset=bass.IndirectOffsetOnAxis(
                                ap=idx_t[:, gs + s:gs + s + 1], axis=0))

                    lo32 = gt[:, :, 0:D2]          # [n, 64] = g00|g10
                    hi32 = gt[:, :, D2:2 * D2]     # [n, 64] = g01|g11

                    if fp16:
                        lo16 = lp.tile([P, n_g, D2], CDT, tag="lo16")
                        nc.scalar.copy(lo16[:], lo32)
                        lo = lo16[:]
                    else:
                        lo = lo32
                    dd = tp.tile([P, n_g, D2], CDT, tag="dd")
                    if fp16 and hi16:
                        hi_t = tp.tile([P, n_g, D2], CDT, tag="t", name="hi_t")
                        nc.scalar.copy(hi_t[:], hi32)
                        nc.vector.tensor_tensor(out=dd[:], in0=hi_t[:],
                                                in1=lo, op=Alu.subtract)
                    else:
                        nc.vector.tensor_tensor(out=dd[:], in0=hi32, in1=lo32,
                                                op=Alu.subtract)

                    if expand_act:
                        txe = ep.tile([P, n_g, 2, C], CDT, tag="txe")
                        nc.scalar.copy(txe[:], tx_c[:, gs:ge, None]
                                       .to_broadcast([P, n_g, 2, C]))
                        txe_v = txe[:]
                        tye = ep.tile([P, n_g, C], CDT, tag="tye")
                        nc.scalar.copy(tye[:], ty_c[:, gs:ge, None]
                                       .to_broadcast([P, n_g, C]))
                        tye_v = tye[:]
                        axe = ep.tile([P, n_g, C], CDT, tag="axe")
                        nc.scalar.copy(axe[:], ax_c[:, gs:ge, None]
                                       .to_broadcast([P, n_g, C]))
                        axe_v = axe[:]
                        aye = ep.tile([P, n_g, C], CDT, tag="aye")
                        nc.scalar.copy(aye[:], ay_c[:, gs:ge, None]
                                       .to_broadcast([P, n_g, C]))
                        aye_v = aye[:]
                        czxe = ep.tile([P, n_g, C], CDT, tag="czxe")
                        nc.scalar.copy(czxe[:], czx_c[:, gs:ge, None]
                                       .to_broadcast([P, n_g, C]))
                        czxe_v = czxe[:]
                        czye = ep.tile([P, n_g, C], CDT, tag="czye")
                        nc.scalar.copy(czye[:], czy_c[:, gs:ge, None]
                                       .to_broadcast([P, n_g, C]))
                        czye_v = czye[:]
                    else:
                        txe_v = tx_c[:, gs:ge, None].to_broadcast([P, n_g, 2, C])
                        tye_v = ty_c[:, gs:ge, None].to_broadcast([P, n_g, C])
                        axe_v = ax_c[:, gs:ge, None].to_broadcast([P, n_g, C])
                        aye_v = ay_c[:, gs:ge, None].to_broadcast([P, n_g, C])
                        czxe_v = czx_c[:, gs:ge, None].to_broadcast([P, n_g, C])
                        czye_v = czy_c[:, gs:ge, None].to_broadcast([P, n_g, C])

                    t_t = tp.tile([P, n_g, 2, C], CDT, tag="t")
                    nc.vector.tensor_tensor(
                        out=t_t[:],
                        in0=dd[:].rearrange("p n (u c) -> p n u c", u=2),
                        in1=txe_v, op=Alu.mult)
                    ab = tp.tile([P, n_g, D2], CDT, tag="ab")
                    nc.vector.tensor_tensor(
                        out=ab[:], in0=lo,
                        in1=t_t[:].rearrange("p n u c -> p n (u c)"),
                        op=Alu.add)
                    dy = tp.tile([P, n_g, C], CDT, tag="dy")
                    nc.vector.tensor_tensor(out=dy[:], in0=ab[:, :, C:D2],
                                            in1=ab[:, :, 0:C],
                                            op=Alu.subtract)
                    tmp = tp.tile([P, n_g, C], CDT, tag="tmp")
                    nc.vector.tensor_tensor(out=tmp[:], in0=dy[:], in1=tye_v,
                                            op=Alu.mult)
                    phi_o = osb[0][:, :, rs:re].rearrange("p c s -> p s c")
                    if act_out:
                        phi_t = tp.tile([P, n_g, C], CDT, tag="t4", name="phi_t")
                        nc.vector.tensor_tensor(out=phi_t[:], in0=ab[:, :, 0:C],
                                                in1=tmp[:], op=Alu.add)
                        nc.scalar.copy(phi_o, phi_t[:])
                    else:
                        nc.vector.tensor_tensor(   # phi
                            out=phi_o, in0=ab[:, :, 0:C], in1=tmp[:],
                            op=Alu.add)
                    eng = nc.gpsimd if pool_ops else nc.vector
                    e_t = tp.tile([P, n_g, C], CDT, tag="e")
                    eng.tensor_tensor(out=e_t[:], in0=dd[:, :, C:D2],
                                      in1=dd[:, :, 0:C], op=Alu.subtract)
                    tmp2 = tp.tile([P, n_g, C], CDT, tag="tmp2")
                    eng.tensor_tensor(out=tmp2[:], in0=e_t[:], in1=tye_v,
                                      op=Alu.mult)
                    djx = tp.tile([P, n_g, C], CDT, tag="djx")
                    eng.tensor_tensor(out=djx[:], in0=dd[:, :, 0:C],
                                      in1=tmp2[:], op=Alu.add)
                    nc.vector.tensor_tensor(   # phi_on_xCam
                        out=osb[1][:, :, rs:re].rearrange("p c s -> p s c"),
                        in0=djx[:], in1=axe_v, op=Alu.mult)
                    py_o = osb[2][:, :, rs:re].rearrange("p c s -> p s c")
                    if act_out:
                        py_t = tp.tile([P, n_g, C], CDT, tag="t5", name="py_t")
                        nc.vector.tensor_tensor(out=py_t[:], in0=dy[:],
                                                in1=aye_v, op=Alu.mult)
                        nc.scalar.copy(py_o, py_t[:])
                    else:
                        nc.vector.tensor_tensor(   # phi_on_yCam
                            out=py_o, in0=dy[:], in1=aye_v, op=Alu.mult)
                    t4 = tp.tile([P, n_g, C], CDT, tag="t4")
                    eng.tensor_tensor(out=t4[:], in0=dy[:], in1=czye_v,
                                      op=Alu.mult)
                    t5 = tp.tile([P, n_g, C], CDT, tag="t5")
                    eng.tensor_tensor(out=t5[:], in0=djx[:], in1=czxe_v,
                                      op=Alu.mult)
                    nc.vector.tensor_tensor(   # phi_on_zCam
                        out=osb[3][:, :, rs:re].rearrange("p c s -> p s c"),
                        in0=t4[:], in1=t5[:], op=Alu.add)

                for k in range(4):
                    dview = out[:, k, :].rearrange("c (p s) -> p c s", p=P)[
                        :, :, ci * CHUNK:(ci + 1) * CHUNK]
                    nc.gpsimd.dma_start(dview, osb[k][:])  # SWDGE fp16->f32

    nc.compile()
    return nc


def _make_core_inputs(inp_b, grid_b, fw_b, fh_b):
    """Host-side shard/layout prep for one core (no arithmetic on values)."""
    feat = np.ascontiguousarray(inp_b.transpose(1, 2, 0))      # [H, W, C]
    fj1 = np.concatenate([feat[:, 1:], feat[:, -1:]], axis=1)  # j+1 (edge dup)
    feat2 = np.concatenate([feat[:-1], feat[1:], fj1[:-1], fj1[1:]], axis=2)
    Hh, Ww, Cc = feat.shape
    return {
        "feat2": feat2.reshape((Hh - 1) * Ww, 4 * Cc),
        "grid_q": np.ascontiguousarray(grid_b, dtype=np.float32),
        "fsw": np.array([[fw_b]], dtype=np.float32),
        "fsh": np.array([[fh_b]], dtype=np.float32),
    }


_CACHED_NC = None


def kernel(input, grid, fScaleWidth, fScaleHeight):
    global _CACHED_NC
    input = np.ascontiguousarray(input, dtype=np.float32)
    grid = np.ascontiguousarray(grid, dtype=np.float32)
    fScaleWidth = np.asarray(fScaleWidth, dtype=np.float32)
    fScaleHeight = np.asarray(fScaleHeight, dtype=np.float32)

    if _CACHED_NC is None:
        _CACHED_NC = _build_kernel()
    nc = _CACHED_NC

    in_maps = []
    for core in range(NCORES):
        b, half = core // 2, core % 2
        in_maps.append(_make_core_inputs(
            input[b], grid[b, half * QC:(half + 1) * QC],
            fScaleWidth[b], fScaleHeight[b]))

    from concourse import bass_utils
    res = bass_utils.run_bass_kernel_spmd(
        nc, in_maps, core_ids=list(range(NCORES)))

    output = np.empty((B, C, 4, Q), np.float32)
    for core in range(NCORES):
        b, half = core // 2, core % 2
        output[b, :, :, half * QC:(half + 1) * QC] = res.results[core]["out"]
    return output

